# revision 41
# baseline (speedup 1.0000x reference)
"""Depthwise box-average (2r+1)x(2r+1), SAME zero padding, on trn2 x8 cores.

Input  x: [4, 32, 512, 512] f32, r = 4 (box 9x9, weights 1/81).
Output same shape.

Strategy (pure data parallel, no collectives):
  * Flatten N*C = 128 images of [512, 512]; core c takes images [16c, 16c+16).
  * Separable filter per image:
      vertical 9-tap:  TensorE banded-Toeplitz matmul (weights 1/81) into PSUM.
        Image rows are split into 4 chunks of 128 (the partition dim).  The
        band crossing chunk boundaries is handled with two sliver matmuls
        (U = last r rows of previous chunk, L = first r rows of next chunk)
        accumulating into the same PSUM bank.  Zero 'SAME' padding at image
        top/bottom is automatic (the band just truncates).
      horizontal 9-tap: sliding-window recurrence
          s[u] = s[u-1] + y[u+r] - y[u-r-1]
        done in one VectorE tensor_tensor_scan per row-chunk (plus two tiny
        edge scans reading a zero tile), after ScalarE evacuates PSUM->SBUF.
  * One 1MiB DMA in and one 1MiB DMA out per image.
"""

import numpy as np

import concourse.bacc as bacc
import concourse.bass as bass
import concourse.mybir as mybir
from concourse.bass_utils import run_bass_kernel_spmd
from concourse.tile import TileContext

N, C, H, W = 4, 32, 512, 512
P = 128                      # SBUF partitions / chunk height
NCORES = 8
IMGS = N * C                 # 128 images total
IMGS_PER_CORE = IMGS // NCORES

F32 = mybir.dt.float32
F32R = mybir.dt.float32r   # TF32-like PE mode: full-rate fp32 matmul, ~1e-6 err
ADD = mybir.AluOpType.add
SUB = mybir.AluOpType.subtract

# set by test harness to capture a profile
TRACE = False
LAST_EXEC_NS = None
LAST_RESULTS = None


def _bands(r: int):
    """Banded Toeplitz blocks for the vertical pass, pre-scaled by 1/(2r+1)^2.

    All are [P, P] so the moving operand is always a full 128-partition chunk
    (matmul requires base partition 0/32/64).  D couples a chunk to itself,
    U couples the previous chunk (rows i ~ global i-P), L the next (i+P).
    """
    k = 2 * r + 1
    w = np.float32(1.0 / (k * k))
    i = np.arange(P)
    D = (np.abs(i[:, None] - i[None, :]) <= r).astype(np.float32) * w
    U = (np.abs((i[:, None] - P) - i[None, :]) <= r).astype(np.float32) * w
    L = (np.abs((i[:, None] + P) - i[None, :]) <= r).astype(np.float32) * w
    return D, U, L


IMGS_PER_DMA = 1          # images batched per DMA transfer
YS_BUFS = 6               # ys pool slots (pads pre-zeroed once per slot)


def _build(r: int, n_imgs: int):
    k = 2 * r + 1
    chunks = H // P
    assert n_imgs % IMGS_PER_DMA == 0
    nc = bacc.Bacc("TRN2", target_bir_lowering=False, debug=False,
                   num_devices=NCORES)

    xsf = nc.dram_tensor("xs", [n_imgs * H, W], F32R, kind="ExternalInput").ap()
    bD = nc.dram_tensor("bD", [P, P], F32R, kind="ExternalInput").ap()
    bU = nc.dram_tensor("bU", [P, P], F32R, kind="ExternalInput").ap()
    bL = nc.dram_tensor("bL", [P, P], F32R, kind="ExternalInput").ap()
    outf = nc.dram_tensor("out", [n_imgs * H, W], F32, kind="ExternalOutput").ap()

    OW = W + r        # output tile row width: r leading pad cols + W data cols
    G = IMGS_PER_DMA
    segs = G * chunks             # row-chunks per DMA group
    nbg = G * chunks - 1          # 128-row boundaries per group (incl unused)

    with TileContext(nc) as tc:
        with (
            tc.tile_pool(name="const", bufs=1) as cpool,
            tc.tile_pool(name="x", bufs=3) as xpool,
            tc.tile_pool(name="ys", bufs=YS_BUFS) as ypool,
            tc.tile_pool(name="o", bufs=3) as opool,
            tc.tile_pool(name="ps", bufs=6, space="PSUM") as ppool,
        ):
            bD_t = cpool.tile([P, P], F32R)
            nc.sync.dma_start(out=bD_t[:], in_=bD)
            bU_t = cpool.tile([P, P], F32R)
            nc.sync.dma_start(out=bU_t[:], in_=bU)
            bL_t = cpool.tile([P, P], F32R)
            nc.sync.dma_start(out=bL_t[:], in_=bL)
            zt = cpool.tile([P, k], F32)
            nc.vector.memset(zt[:], 0.0)
            # pre-zero the pad columns of every ys slot once; the per-chunk
            # copies only ever write cols [k, k+W), so the pads stay zero as
            # the slots rotate.
            YSW = k + W + r
            for _ in range(YS_BUFS):
                ysi = ypool.tile([P, YSW], F32, tag="ys")
                nc.vector.memset(ysi[:, 0:k], 0.0)
                nc.vector.memset(ysi[:, k + W:YSW], 0.0)
            # dummy activation so the ~2.7us ACT_TABLE_LOAD overlaps the
            # first input DMA instead of sitting on the critical path
            warm = cpool.tile([1, 1], F32)
            nc.scalar.copy(out=warm[:], in_=zt[0:1, 0:1])

            for g in range(n_imgs // G):
                row0 = g * G * H
                xt = xpool.tile([P, segs, W], F32R, tag="x")
                xin = xsf[row0:row0 + G * H].rearrange("(s p) w -> p s w", p=P)
                # chunk-granular loads: downstream matmuls/scans start as
                # soon as each 256KB lands
                for s0 in range(segs):
                    nc.sync.dma_start(out=xt[:, s0, :], in_=xin[:, s0, :])
                ot = opool.tile([P, segs, OW], F32, tag="o")
                for u in range(G):
                    for j in range(chunks):
                        s = u * chunks + j
                        y = ppool.tile([P, W], F32, tag="y")
                        n_mm = 1 + (j > 0) + (j < chunks - 1)
                        nc.tensor.matmul(y[:], bD_t[:], xt[:, s, :],
                                         start=True, stop=(n_mm == 1))
                        done = 1
                        if j > 0:
                            done += 1
                            nc.tensor.matmul(y[:], bU_t[:], xt[:, s - 1, :],
                                             start=False, stop=(done == n_mm))
                        if j < chunks - 1:
                            done += 1
                            nc.tensor.matmul(y[:], bL_t[:], xt[:, s + 1, :],
                                             start=False, stop=(done == n_mm))

                        # ys = [k zeros][y 0..W-1][r zeros] (pads pre-zeroed
                        # per slot above).  The single scan
                        #   s[u] = (y[u+r] + s[u-1]) - y[u-r-1],  u in [-r, W)
                        # reads both shifted views from ys and writes the
                        # whole padded output row in one DVE op.
                        ys = ypool.tile([P, YSW], F32, tag="ys")
                        nc.scalar.copy(out=ys[:, k:k + W], in_=y[:])

                        oj = ot[:, s, :]
                        nc.vector.tensor_tensor_scan(
                            oj[:, 0:W + r], ys[:, k:k + W + r], ys[:, 0:W + r],
                            0.0, ADD, SUB)

                # out-DMA on the gpsimd SWDGE queue so its semaphore wait
                # doesn't head-of-line-block the input DMAs on the sync
                # queue; half-image granularity so draining starts earlier
                oo = outf[row0:row0 + G * H].rearrange("(s p) w -> p s w", p=P)
                h2 = segs // 2
                nc.gpsimd.dma_start(out=oo[:, 0:h2, :],
                                    in_=ot[:, 0:h2, r:r + W])
                nc.gpsimd.dma_start(out=oo[:, h2:segs, :],
                                    in_=ot[:, h2:segs, r:r + W])
    nc.compile()
    return nc


def kernel(x, r):
    global LAST_EXEC_NS, LAST_RESULTS
    x = np.ascontiguousarray(np.asarray(x, dtype=np.float32))
    r = int(r)
    assert x.shape == (N, C, H, W)
    assert 1 <= r < P and H % P == 0

    D, U, L = _bands(r)
    nc = _build(r, IMGS_PER_CORE)

    shards = x.reshape(NCORES, IMGS_PER_CORE * H, W)
    in_maps = [
        {"xs": np.ascontiguousarray(shards[c]), "bD": D, "bU": U, "bL": L}
        for c in range(NCORES)
    ]
    res = run_bass_kernel_spmd(
        nc, in_maps, core_ids=list(range(NCORES)), trace=TRACE)
    LAST_EXEC_NS = res.exec_time_ns
    LAST_RESULTS = res
    outs = np.stack([res.results[c]["out"] for c in range(NCORES)], axis=0)
    return outs.reshape(N, C, H, W)


# revision 42
# speedup vs baseline: 1.0341x; 1.0341x over previous
"""Depthwise box-average (2r+1)x(2r+1), SAME zero padding, on trn2 x8 cores.

Input  x: [4, 32, 512, 512] f32, r = 4 (box 9x9, weights 1/81).
Output same shape.

Strategy (pure data parallel, no collectives):
  * Flatten N*C = 128 images of [512, 512]; core c takes images [16c, 16c+16).
  * Separable filter per image:
      vertical 9-tap:  TensorE banded-Toeplitz matmul (weights 1/81) into PSUM.
        Image rows are split into 4 chunks of 128 (the partition dim).  The
        band crossing chunk boundaries is handled with two sliver matmuls
        (U = last r rows of previous chunk, L = first r rows of next chunk)
        accumulating into the same PSUM bank.  Zero 'SAME' padding at image
        top/bottom is automatic (the band just truncates).
      horizontal 9-tap: sliding-window recurrence
          s[u] = s[u-1] + y[u+r] - y[u-r-1]
        done in one VectorE tensor_tensor_scan per row-chunk (plus two tiny
        edge scans reading a zero tile), after ScalarE evacuates PSUM->SBUF.
  * One 1MiB DMA in and one 1MiB DMA out per image.
"""

import numpy as np

import concourse.bacc as bacc
import concourse.bass as bass
import concourse.mybir as mybir
from concourse.bass_utils import run_bass_kernel_spmd
from concourse.tile import TileContext

N, C, H, W = 4, 32, 512, 512
P = 128                      # SBUF partitions / chunk height
NCORES = 8
IMGS = N * C                 # 128 images total
IMGS_PER_CORE = IMGS // NCORES

F32 = mybir.dt.float32
F32R = mybir.dt.float32r   # TF32-like PE mode: full-rate fp32 matmul, ~1e-6 err
ADD = mybir.AluOpType.add
SUB = mybir.AluOpType.subtract

# set by test harness to capture a profile
TRACE = False
LAST_EXEC_NS = None
LAST_RESULTS = None


def _bands(r: int):
    """Banded Toeplitz blocks for the vertical pass, pre-scaled by 1/(2r+1)^2.

    All are [P, P] so the moving operand is always a full 128-partition chunk
    (matmul requires base partition 0/32/64).  D couples a chunk to itself,
    U couples the previous chunk (rows i ~ global i-P), L the next (i+P).
    """
    k = 2 * r + 1
    w = np.float32(1.0 / (k * k))
    i = np.arange(P)
    D = (np.abs(i[:, None] - i[None, :]) <= r).astype(np.float32) * w
    U = (np.abs((i[:, None] - P) - i[None, :]) <= r).astype(np.float32) * w
    L = (np.abs((i[:, None] + P) - i[None, :]) <= r).astype(np.float32) * w
    return D, U, L


IMGS_PER_DMA = 1          # images batched per DMA transfer
YS_BUFS = 6               # ys pool slots (pads pre-zeroed once per slot)


def _build(r: int, n_imgs: int):
    k = 2 * r + 1
    chunks = H // P
    assert n_imgs % IMGS_PER_DMA == 0
    nc = bacc.Bacc("TRN2", target_bir_lowering=False, debug=False,
                   num_devices=NCORES)

    xsf = nc.dram_tensor("xs", [n_imgs * H, W], F32R, kind="ExternalInput").ap()
    bD = nc.dram_tensor("bD", [P, P], F32R, kind="ExternalInput").ap()
    bU = nc.dram_tensor("bU", [P, P], F32R, kind="ExternalInput").ap()
    bL = nc.dram_tensor("bL", [P, P], F32R, kind="ExternalInput").ap()
    outf = nc.dram_tensor("out", [n_imgs * H, W], F32, kind="ExternalOutput").ap()

    OW = W + r        # output tile row width: r leading pad cols + W data cols
    G = IMGS_PER_DMA
    segs = G * chunks             # row-chunks per DMA group
    nbg = G * chunks - 1          # 128-row boundaries per group (incl unused)

    with TileContext(nc) as tc:
        with (
            tc.tile_pool(name="const", bufs=1) as cpool,
            tc.tile_pool(name="x", bufs=3) as xpool,
            tc.tile_pool(name="ys", bufs=YS_BUFS) as ypool,
            tc.tile_pool(name="o", bufs=3) as opool,
            tc.tile_pool(name="ps", bufs=6, space="PSUM") as ppool,
        ):
            bD_t = cpool.tile([P, P], F32R)
            nc.sync.dma_start(out=bD_t[:], in_=bD)
            bU_t = cpool.tile([P, P], F32R)
            nc.sync.dma_start(out=bU_t[:], in_=bU)
            bL_t = cpool.tile([P, P], F32R)
            nc.sync.dma_start(out=bL_t[:], in_=bL)
            zt = cpool.tile([P, k], F32)
            nc.vector.memset(zt[:], 0.0)
            # pre-zero the pad columns of every ys slot once; the per-chunk
            # copies only ever write cols [k, k+W), so the pads stay zero as
            # the slots rotate.
            YSW = k + W + r
            for _ in range(YS_BUFS):
                ysi = ypool.tile([P, YSW], F32, tag="ys")
                nc.vector.memset(ysi[:, 0:k], 0.0)
                nc.vector.memset(ysi[:, k + W:YSW], 0.0)
            # dummy activation so the ~2.7us ACT_TABLE_LOAD overlaps the
            # first input DMA instead of sitting on the critical path
            warm = cpool.tile([1, 1], F32)
            nc.scalar.copy(out=warm[:], in_=zt[0:1, 0:1])

            for g in range(n_imgs // G):
                row0 = g * G * H
                xt = xpool.tile([P, segs, W], F32R, tag="x")
                xin = xsf[row0:row0 + G * H].rearrange("(s p) w -> p s w", p=P)
                if g == 0:
                    # split the very first load per chunk so the pipeline
                    # fills as soon as the first 256KB lands
                    for s0 in range(segs):
                        nc.sync.dma_start(out=xt[:, s0, :], in_=xin[:, s0, :])
                else:
                    # half-image granularity: downstream matmuls/scans can
                    # start when the first half lands
                    h1 = segs // 2
                    nc.sync.dma_start(out=xt[:, 0:h1, :], in_=xin[:, 0:h1, :])
                    nc.sync.dma_start(out=xt[:, h1:segs, :],
                                      in_=xin[:, h1:segs, :])
                ot = opool.tile([P, segs, OW], F32, tag="o")
                for u in range(G):
                    for j in range(chunks):
                        s = u * chunks + j
                        y = ppool.tile([P, W], F32, tag="y")
                        n_mm = 1 + (j > 0) + (j < chunks - 1)
                        nc.tensor.matmul(y[:], bD_t[:], xt[:, s, :],
                                         start=True, stop=(n_mm == 1))
                        done = 1
                        if j > 0:
                            done += 1
                            nc.tensor.matmul(y[:], bU_t[:], xt[:, s - 1, :],
                                             start=False, stop=(done == n_mm))
                        if j < chunks - 1:
                            done += 1
                            nc.tensor.matmul(y[:], bL_t[:], xt[:, s + 1, :],
                                             start=False, stop=(done == n_mm))

                        # ys = [k zeros][y 0..W-1][r zeros] (pads pre-zeroed
                        # per slot above).  The single scan
                        #   s[u] = (y[u+r] + s[u-1]) - y[u-r-1],  u in [-r, W)
                        # reads both shifted views from ys and writes the
                        # whole padded output row in one DVE op.
                        ys = ypool.tile([P, YSW], F32, tag="ys")
                        nc.scalar.copy(out=ys[:, k:k + W], in_=y[:])

                        oj = ot[:, s, :]
                        nc.vector.tensor_tensor_scan(
                            oj[:, 0:W + r], ys[:, k:k + W + r], ys[:, 0:W + r],
                            0.0, ADD, SUB)

                # out-DMA on the gpsimd SWDGE queue so its semaphore wait
                # doesn't head-of-line-block the input DMAs on the sync
                # queue; half-image granularity so draining starts earlier
                oo = outf[row0:row0 + G * H].rearrange("(s p) w -> p s w", p=P)
                h2 = segs // 2
                nc.gpsimd.dma_start(out=oo[:, 0:h2, :],
                                    in_=ot[:, 0:h2, r:r + W])
                nc.gpsimd.dma_start(out=oo[:, h2:segs, :],
                                    in_=ot[:, h2:segs, r:r + W])
    nc.compile()
    return nc


def kernel(x, r):
    global LAST_EXEC_NS, LAST_RESULTS
    x = np.ascontiguousarray(np.asarray(x, dtype=np.float32))
    r = int(r)
    assert x.shape == (N, C, H, W)
    assert 1 <= r < P and H % P == 0

    D, U, L = _bands(r)
    nc = _build(r, IMGS_PER_CORE)

    shards = x.reshape(NCORES, IMGS_PER_CORE * H, W)
    in_maps = [
        {"xs": np.ascontiguousarray(shards[c]), "bD": D, "bU": U, "bL": L}
        for c in range(NCORES)
    ]
    res = run_bass_kernel_spmd(
        nc, in_maps, core_ids=list(range(NCORES)), trace=TRACE)
    LAST_EXEC_NS = res.exec_time_ns
    LAST_RESULTS = res
    outs = np.stack([res.results[c]["out"] for c in range(NCORES)], axis=0)
    return outs.reshape(N, C, H, W)
